# revision 12
# baseline (speedup 1.0000x reference)
"""Attention pooling kernel for Trainium2 (8 NeuronCores).

Reference computation (per batch b):
    score   = tanh(x @ W + b)          # (S, 1)
    weights = softmax(score, axis=seq) # (S, 1)
    context = sum(x * weights, axis=seq)  # (D,)

Sharding: data-parallel over batch (32 batches -> 4 per core).

Strategy per core (compute in bf16 on the TensorEngine, f32 accumulation):
  - Host converts x to bf16 and ships BOTH layouts (seq-major for the
    context matmul, dim-major for the score matmul) pre-swizzled into
    128-partition tiles so every load is a fully contiguous DMA.
  - score: lhsT = W d-chunk [128,1], rhs = xT [128d, 512s] -> psum [1, 512]
    accumulated over 4 d-chunks -> score lives as [1, S] on partition 0.
  - softmax entirely in the [1, S] layout: tanh is bounded so no
    max-subtraction is needed; exp uses activation accum_out for the sum,
    so no cross-partition reduce is needed either.
  - weights output is written straight from the f32 [1, S] tile.
  - bf16 weights roundtrip through DRAM + xbar-transpose DMA to land as
    [128, 32] (seq on partitions) = the lhsT columns for the context matmul.
  - context: lhsT = weight column [128,1], rhs = x_native [128s, 512d]
    -> psum [1, 512] accumulated over 32 s-tiles.
  - PE instructions are hardware-decoded and only carry a single sync wait:
    tiny "absorber" matmuls take the DMA-completion waits so every real
    matmul needs at most one.
"""

import sys

for p in ("/opt/trn_rl_repo",):
    if p not in sys.path:
        sys.path.insert(0, p)

import numpy as np
import ml_dtypes

B, S, D = 32, 4096, 512
NCORES = 8
BPC = B // NCORES  # batches per core
ST = S // 128      # 32 seq tiles
DC = D // 128      # 4 dim chunks

_cache = {}


def _build(fix_waits=True):
    import concourse.bass as bass
    import concourse.mybir as mybir
    from concourse import tile
    from concourse.tile_rust import add_dep_helper

    dt = mybir.dt
    AF = mybir.ActivationFunctionType

    nc = bass.Bass()
    xn = nc.declare_dram_parameter("xn", [BPC, 128, ST, D], dt.bfloat16, isOutput=False)
    xt = nc.declare_dram_parameter("xt", [BPC, 128, DC, S], dt.bfloat16, isOutput=False)
    wv = nc.declare_dram_parameter("wv", [128, DC], dt.bfloat16, isOutput=False)
    bf = nc.declare_dram_parameter("bf", [1, S], dt.float32, isOutput=False)
    ctx_out = nc.declare_dram_parameter("ctx_out", [BPC, D], dt.float32, isOutput=True)
    w_out = nc.declare_dram_parameter("w_out", [BPC, S], dt.float32, isOutput=True)

    with tile.TileContext(nc) as tc:
        with (
            tc.tile_pool(name="xpool", bufs=2) as xpool,
            tc.tile_pool(name="consts", bufs=1) as cpool,
            tc.tile_pool(name="small", bufs=2) as spool,
            tc.tile_pool(name="psum_sc", bufs=2, space="PSUM") as psc,
            tc.tile_pool(name="psum_ctx", bufs=2, space="PSUM") as pctx,
            tc.tile_pool(name="psum_dmy", bufs=1, space="PSUM") as pdmy,
            tc.tile_pool(name="dram", bufs=2, space="DRAM") as dpool,
        ):
            wv_sb = cpool.tile([128, DC], dt.bfloat16, tag="wv")
            nc.sync.dma_start(out=wv_sb[:], in_=wv[:])
            bf_sb = cpool.tile([1, S], dt.float32, tag="bf")
            nc.sync.dma_start(out=bf_sb[:], in_=bf[:])

            def absorb(lhsT, rhs):
                """Tiny matmul whose only job is to carry a DMA-completion
                wait, so the real (HW-decoded, single-wait-slot) matmuls
                that follow don't have to."""
                dmy = pdmy.tile([1, 1], dt.float32, tag="dmy")
                return nc.tensor.matmul(dmy[:], lhsT, rhs, start=True, stop=True)

            absorb(wv_sb[:, 0:1], wv_sb[:, 0:1])

            for b in range(BPC):
                xn_sb = xpool.tile([128, ST * D], dt.bfloat16, tag="xn")
                nc.sync.dma_start(out=xn_sb[:], in_=xn[b].rearrange("p t d -> p (t d)"))
                xt_sb = xpool.tile([128, DC * S], dt.bfloat16, tag="xt")
                nc.sync.dma_start(out=xt_sb[:], in_=xt[b].rearrange("p c s -> p (c s)"))
                a_xn = absorb(xn_sb[:, 0:1], xn_sb[:, 0:1])
                a_xt = absorb(xt_sb[:, 0:1], xt_sb[:, 0:1])

                # ---- score = x @ W + b, laid out [1, S] on partition 0 ----
                sc_sb = spool.tile([1, S], dt.float32, tag="sc")
                for n in range(4):
                    ps = psc.tile([1, 1024], dt.float32, tag="ps")
                    for h in range(2):
                        for j in range(DC):
                            col = n * 1024 + h * 512
                            mm = nc.tensor.matmul(
                                ps[:, h * 512 : (h + 1) * 512],
                                wv_sb[:, j : j + 1],
                                xt_sb[:, j * S + col : j * S + col + 512],
                                start=(j == 0),
                                stop=(j == DC - 1),
                            )
                            if n == 0 and h == 0 and j == 0:
                                add_dep_helper(mm.ins, a_xt.ins, False, "order after absorber")
                    # fused psum->sbuf copy + bias add
                    nc.vector.tensor_add(
                        sc_sb[:, n * 1024 : (n + 1) * 1024],
                        ps[:],
                        bf_sb[:, n * 1024 : (n + 1) * 1024],
                    )

                # ---- softmax on [1, S], in place: w = exp(tanh(sc)) / sum ----
                nc.scalar.activation(sc_sb[:], sc_sb[:], AF.Tanh)
                esum = spool.tile([1, 1], dt.float32, tag="esum")
                nc.scalar.activation(sc_sb[:], sc_sb[:], AF.Exp, accum_out=esum[:])
                rec = spool.tile([1, 1], dt.float32, tag="rec")
                nc.vector.reciprocal(rec[:], esum[:])
                nc.scalar.mul(sc_sb[:], sc_sb[:], rec[:])

                # weights output: already contiguous in [1, S]
                nc.sync.dma_start(out=w_out[b].unsqueeze(0), in_=sc_sb[:])

                # ---- bf16 weights -> DRAM -> xbar transpose -> [128, ST] ----
                w_bf = spool.tile([1, S], dt.bfloat16, tag="wbf")
                nc.vector.tensor_copy(w_bf[:], sc_sb[:])
                wd = dpool.tile([ST, 128], dt.bfloat16, tag="wd")
                nc.sync.dma_start(
                    out=wd[:].rearrange("a b -> (a b)").unsqueeze(0), in_=w_bf[:]
                )
                w_col = spool.tile([128, ST], dt.bfloat16, tag="wcol")
                nc.sync.dma_start(out=w_col[:], in_=wd[:], transpose=True)
                a_wc = absorb(w_col[:, 0:1], w_col[:, 0:1])

                # ---- context = sum_s w[s] * x[s, :] ----
                pc = pctx.tile([1, D], dt.float32, tag="pc")
                for i in range(ST):
                    mm = nc.tensor.matmul(
                        pc[:],
                        w_col[:, i : i + 1],
                        xn_sb[:, i * D : (i + 1) * D],
                        start=(i == 0),
                        stop=(i == ST - 1),
                    )
                    if i == 0:
                        add_dep_helper(mm.ins, a_wc.ins, False, "order after absorber")
                        add_dep_helper(mm.ins, a_xn.ins, False, "order after absorber")
                ctx_sb = spool.tile([1, D], dt.float32, tag="ctx")
                nc.scalar.copy(ctx_sb[:], pc[:])
                nc.sync.dma_start(out=ctx_out[b : b + 1, :], in_=ctx_sb[:])

    if fix_waits:
        _fix_pe_waits(nc, mybir)
    return nc


def _fix_pe_waits(nc, mybir):
    """Engine instructions hold a single hardware sync-wait slot; Tile
    sometimes emits 2+ waits on one instruction (psum/tile slot reuse), which
    walrus rejects with 'Too many sync wait commands'.  Splice standalone
    EventSemaphore instructions (one wait each) into the same engine queue
    immediately before each over-subscribed instruction — semantically
    identical, the sequencer just waits in two steps."""
    f = nc.m.functions[0]
    counter = [0]
    for blk in f.blocks:
        insts = list(blk.instructions)
        out = []
        changed = False
        for inst in insts:
            si = inst.sync_info
            nw = len(si.on_wait) if si is not None and si.on_wait else 0
            if nw > 1:
                waits = list(si.on_wait)
                for w in waits[:-1]:
                    es = mybir.InstEventSemaphore(
                        name=f"I-eswait-{counter[0]}", ins=[], outs=[]
                    )
                    counter[0] += 1
                    es.engine = inst.engine
                    es.sync_info = mybir.SyncInfo(on_wait=[w], on_update=[])
                    out.append(es)
                si.on_wait = waits[-1:]
                changed = True
            out.append(inst)
        if changed:
            blk.instructions = out


def _prep_inputs(x, W, b):
    bf16 = ml_dtypes.bfloat16
    xbf = x.astype(bf16)  # (B, S, D)
    # native, pre-swizzled: xn[b, p, t, d] = x[b, t*128+p, d]
    xn = np.ascontiguousarray(xbf.reshape(B, ST, 128, D).transpose(0, 2, 1, 3))
    # transposed, pre-swizzled: xt[b, p, c, s] = x[b, s, c*128+p]
    xt = np.ascontiguousarray(xbf.reshape(B, S, DC, 128).transpose(0, 3, 2, 1))
    wv = np.ascontiguousarray(
        W.reshape(DC, 128).T.astype(bf16)
    )  # [128, DC], col j = W[j*128:(j+1)*128]
    bfl = np.ascontiguousarray(b.reshape(1, S).astype(np.float32))
    in_maps = []
    for c in range(NCORES):
        lo = c * BPC
        in_maps.append(
            {
                "xn": xn[lo : lo + BPC],
                "xt": xt[lo : lo + BPC],
                "wv": wv,
                "bf": bfl,
            }
        )
    return in_maps


def kernel(x, W, b):
    from concourse.bass_utils import run_bass_kernel_spmd

    x = np.asarray(x, dtype=np.float32)
    W = np.asarray(W, dtype=np.float32)
    b = np.asarray(b, dtype=np.float32)

    if "nc" not in _cache:
        _cache["nc"] = _build()
    nc = _cache["nc"]

    in_maps = _prep_inputs(x, W, b)
    res = run_bass_kernel_spmd(nc, in_maps, list(range(NCORES))).results

    context = np.concatenate(
        [np.asarray(res[c]["ctx_out"], dtype=np.float32) for c in range(NCORES)], axis=0
    )  # (B, D)
    weights = np.concatenate(
        [np.asarray(res[c]["w_out"], dtype=np.float32) for c in range(NCORES)], axis=0
    ).reshape(B, S, 1)
    return context, weights


# revision 15
# speedup vs baseline: 1.4294x; 1.4294x over previous
"""Attention pooling kernel for Trainium2 (8 NeuronCores).

Reference computation (per batch b):
    score   = tanh(x @ W + b)          # (S, 1)
    weights = softmax(score, axis=seq) # (S, 1)
    context = sum(x * weights, axis=seq)  # (D,)

Sharding: data-parallel over batch (32 batches -> 4 per core).

Strategy per core (compute in bf16 on the TensorEngine, f32 accumulation):
  - Host converts x to bf16 and ships BOTH layouts (seq-major for the
    context matmul, dim-major for the score matmul) pre-swizzled into
    128-partition tiles so every load is a fully contiguous DMA.
  - score: lhsT = W d-chunk [128,1], rhs = xT [128d, 512s] -> psum [1, 512]
    accumulated over 4 d-chunks -> score lives as [1, S] on partition 0.
  - softmax entirely in the [1, S] layout: tanh is bounded so no
    max-subtraction is needed; exp uses activation accum_out for the sum,
    so no cross-partition reduce is needed either.
  - weights output is written straight from the f32 [1, S] tile.
  - bf16 weights roundtrip through DRAM + xbar-transpose DMA to land as
    [128, 32] (seq on partitions) = the lhsT columns for the context matmul.
  - context: lhsT = weight column [128,1], rhs = x_native [128s, 512d]
    -> psum [1, 512] accumulated over 32 s-tiles.
  - PE instructions are hardware-decoded and only carry a single sync wait:
    tiny "absorber" matmuls take the DMA-completion waits so every real
    matmul needs at most one.
"""

import sys

for p in ("/opt/trn_rl_repo",):
    if p not in sys.path:
        sys.path.insert(0, p)

import numpy as np
import ml_dtypes

B, S, D = 32, 4096, 512
NCORES = 8
BPC = B // NCORES  # batches per core
ST = S // 128      # 32 seq tiles
DC = D // 128      # 4 dim chunks

_cache = {}


def _build(fix_waits=True):
    import concourse.bass as bass
    import concourse.mybir as mybir
    from concourse import tile

    dt = mybir.dt
    AF = mybir.ActivationFunctionType

    nc = bass.Bass()
    # xt: dim-major halves, [b, w, p, c, 2048]: window w covers seq
    #     [w*2048, (w+1)*2048); element (p, c, s) = x[b, w*2048+s, c*128+p]
    # xn: seq-major halves, [b, g, p, t, 512]: element (p, t, d) =
    #     x[b, (g*16+t)*128 + p, d]
    xt = nc.declare_dram_parameter(
        "xt", [BPC, 2, 128, DC, S // 2], dt.bfloat16, isOutput=False
    )
    xn = nc.declare_dram_parameter(
        "xn", [BPC, 2, 128, ST // 2, D], dt.bfloat16, isOutput=False
    )
    wv = nc.declare_dram_parameter("wv", [128, DC], dt.bfloat16, isOutput=False)
    bf = nc.declare_dram_parameter("bf", [1, S], dt.float32, isOutput=False)
    ctx_out = nc.declare_dram_parameter("ctx_out", [BPC, D], dt.float32, isOutput=True)
    w_out = nc.declare_dram_parameter("w_out", [BPC, S], dt.float32, isOutput=True)

    SH = S // 2  # 2048, seq window size

    with tile.TileContext(nc) as tc:
        with (
            tc.tile_pool(name="xtpool", bufs=4) as xtpool,
            tc.tile_pool(name="xnpool", bufs=4) as xnpool,
            tc.tile_pool(name="consts", bufs=1) as cpool,
            tc.tile_pool(name="small", bufs=2) as spool,
            tc.tile_pool(name="psum_sc", bufs=2, space="PSUM") as psc,
            tc.tile_pool(name="psum_ctx", bufs=2, space="PSUM") as pctx,
            tc.tile_pool(name="dram", bufs=2, space="DRAM") as dpool,
        ):
            wv_sb = cpool.tile([128, DC], dt.bfloat16, tag="wv")
            nc.sync.dma_start(out=wv_sb[:], in_=wv[:])
            bf_sb = cpool.tile([1, S], dt.float32, tag="bf")
            nc.sync.dma_start(out=bf_sb[:], in_=bf[:])

            for b in range(BPC):
                # big loads on the SP HWDGE queue, 2 MiB each, window-granular
                xt_sb = []
                xn_sb = []
                for w in range(2):
                    t = xtpool.tile([128, DC * SH], dt.bfloat16, tag="xtw")
                    nc.sync.dma_start(out=t[:], in_=xt[b, w].rearrange("p c s -> p (c s)"))
                    xt_sb.append(t)
                for g in range(2):
                    t = xnpool.tile([128, (ST // 2) * D], dt.bfloat16, tag="xnw")
                    nc.sync.dma_start(out=t[:], in_=xn[b, g].rearrange("p t d -> p (t d)"))
                    xn_sb.append(t)

                # ---- score = x @ W + b, laid out [1, S] on partition 0 ----
                sc_sb = spool.tile([1, S], dt.float32, tag="sc")
                for n in range(4):
                    w, so = n // 2, (n % 2) * 1024
                    ps = psc.tile([1, 1024], dt.float32, tag="ps")
                    for h in range(2):
                        for j in range(DC):
                            col = so + h * 512
                            nc.tensor.matmul(
                                ps[:, h * 512 : (h + 1) * 512],
                                wv_sb[:, j : j + 1],
                                xt_sb[w][:, j * SH + col : j * SH + col + 512],
                                start=(j == 0),
                                stop=(j == DC - 1),
                            )
                    # fused psum->sbuf copy + bias add
                    nc.vector.tensor_add(
                        sc_sb[:, n * 1024 : (n + 1) * 1024],
                        ps[:],
                        bf_sb[:, n * 1024 : (n + 1) * 1024],
                    )

                # ---- softmax on [1, S], in place: w = exp(tanh(sc)) / sum ----
                nc.scalar.activation(sc_sb[:], sc_sb[:], AF.Tanh)
                esum = spool.tile([1, 1], dt.float32, tag="esum")
                nc.scalar.activation(sc_sb[:], sc_sb[:], AF.Exp, accum_out=esum[:])
                rec = spool.tile([1, 1], dt.float32, tag="rec")
                nc.vector.reciprocal(rec[:], esum[:])
                w_bf = spool.tile([1, S], dt.bfloat16, tag="wbf")
                nc.vector.tensor_scalar_mul(w_bf[:], sc_sb[:], rec[:])
                nc.vector.tensor_scalar_mul(sc_sb[:], sc_sb[:], rec[:])

                # weights output (f32, contiguous); small DMAs ride the
                # ACT HWDGE queue so they don't queue behind the big loads
                nc.scalar.dma_start(out=w_out[b].unsqueeze(0), in_=sc_sb[:])

                # ---- bf16 weights -> DRAM -> xbar transpose -> [128, ST] ----
                wd = dpool.tile([ST, 128], dt.bfloat16, tag="wd")
                nc.scalar.dma_start(
                    out=wd[:].rearrange("a b -> (a b)").unsqueeze(0), in_=w_bf[:]
                )
                w_col = spool.tile([128, ST], dt.bfloat16, tag="wcol")
                nc.scalar.dma_start(out=w_col[:], in_=wd[:], transpose=True)

                # ---- context = sum_s w[s] * x[s, :] ----
                pc = pctx.tile([1, D], dt.float32, tag="pc")
                for i in range(ST):
                    nc.tensor.matmul(
                        pc[:],
                        w_col[:, i : i + 1],
                        xn_sb[i // 16][:, (i % 16) * D : (i % 16 + 1) * D],
                        start=(i == 0),
                        stop=(i == ST - 1),
                    )
                ctx_sb = spool.tile([1, D], dt.float32, tag="ctx")
                nc.scalar.copy(ctx_sb[:], pc[:])
                nc.scalar.dma_start(out=ctx_out[b : b + 1, :], in_=ctx_sb[:])

    if fix_waits:
        _fix_pe_waits(nc, mybir)
    return nc


def _fix_pe_waits(nc, mybir):
    """Engine instructions hold a single hardware sync-wait slot; Tile
    sometimes emits 2+ waits on one instruction (psum/tile slot reuse), which
    walrus rejects with 'Too many sync wait commands'.  Splice standalone
    EventSemaphore instructions (one wait each) into the same engine queue
    immediately before each over-subscribed instruction — semantically
    identical, the sequencer just waits in two steps."""
    f = nc.m.functions[0]
    counter = [0]
    for blk in f.blocks:
        insts = list(blk.instructions)
        out = []
        changed = False
        for inst in insts:
            si = inst.sync_info
            nw = len(si.on_wait) if si is not None and si.on_wait else 0
            if nw > 1:
                waits = list(si.on_wait)
                for w in waits[:-1]:
                    es = mybir.InstEventSemaphore(
                        name=f"I-eswait-{counter[0]}", ins=[], outs=[]
                    )
                    counter[0] += 1
                    es.engine = inst.engine
                    es.sync_info = mybir.SyncInfo(on_wait=[w], on_update=[])
                    out.append(es)
                si.on_wait = waits[-1:]
                changed = True
            out.append(inst)
        if changed:
            blk.instructions = out


def _prep_inputs(x, W, b):
    bf16 = ml_dtypes.bfloat16
    xbf = x.astype(bf16)  # (B, S, D)
    # native, seq-major halves: xn[b, g, p, t, d] = x[b, (g*16+t)*128+p, d]
    xn = np.ascontiguousarray(
        xbf.reshape(B, 2, ST // 2, 128, D).transpose(0, 1, 3, 2, 4)
    )
    # dim-major halves: xt[b, w, p, c, s] = x[b, w*2048+s, c*128+p]
    xt = np.ascontiguousarray(
        xbf.reshape(B, 2, S // 2, DC, 128).transpose(0, 1, 4, 3, 2)
    )
    wv = np.ascontiguousarray(
        W.reshape(DC, 128).T.astype(bf16)
    )  # [128, DC], col j = W[j*128:(j+1)*128]
    bfl = np.ascontiguousarray(b.reshape(1, S).astype(np.float32))
    in_maps = []
    for c in range(NCORES):
        lo = c * BPC
        in_maps.append(
            {
                "xn": xn[lo : lo + BPC],
                "xt": xt[lo : lo + BPC],
                "wv": wv,
                "bf": bfl,
            }
        )
    return in_maps


def kernel(x, W, b):
    from concourse.bass_utils import run_bass_kernel_spmd

    x = np.asarray(x, dtype=np.float32)
    W = np.asarray(W, dtype=np.float32)
    b = np.asarray(b, dtype=np.float32)

    if "nc" not in _cache:
        _cache["nc"] = _build()
    nc = _cache["nc"]

    in_maps = _prep_inputs(x, W, b)
    res = run_bass_kernel_spmd(nc, in_maps, list(range(NCORES))).results

    context = np.concatenate(
        [np.asarray(res[c]["ctx_out"], dtype=np.float32) for c in range(NCORES)], axis=0
    )  # (B, D)
    weights = np.concatenate(
        [np.asarray(res[c]["w_out"], dtype=np.float32) for c in range(NCORES)], axis=0
    ).reshape(B, S, 1)
    return context, weights
